# revision 4
# baseline (speedup 1.0000x reference)
"""Causal bag-of-words kernel for Trainium2 (8 NeuronCores, SPMD).

out[b, t, :] = mean(x[b, :t+1, :], axis=0)  for x of shape (8, 8192, 512) f32.

Sharding: data-parallel over B — core b handles x[b] (8192, 512) independently.

Per-core algorithm (all in natural [t, c] layout, no transposes):
  T = 8192 is split into 64 blocks of 128 rows (partition dim).
  For block k with rows X_k [128, 512]:
    psum_k = U @ X_k + J @ Z_{k-1}      (two accumulating PE matmuls)
  where U is upper-triangular ones (cumsum within the block), J is all-ones
  (broadcasts the column-sum of Z over all 128 rows), and
  Z_{k-1} = sum_{j<k} X_j is a running elementwise block sum maintained with
  one DVE add per block.  The 1/(t+1) scaling is folded into the PSUM->SBUF
  evacuation copy as a per-partition scalar multiply (split DVE/ACT).
  Blocks are streamed in waves of 8 (2 MiB DMAs) and written back the same way.
"""

import sys

sys.path.insert(0, "/opt/trn_rl_repo")

import numpy as np

import concourse.bacc as bacc
import concourse.bass as bass
import concourse.mybir as mybir
import concourse.tile as tile
from concourse.bass_utils import run_bass_kernel_spmd

B, T, C = 8, 8192, 512
P = 128                 # partition dim / block size along T
NB = T // P             # 64 blocks
G = 8                   # blocks per wave (2 MiB per DMA)
NW = NB // G            # 8 waves
N_CORES = 8
F32 = mybir.dt.float32

_cache: dict = {}


def build_program(n_iter: int = 1, loop_n: int = 1):
    """Build + compile the per-core Bass program (SPMD, identical on all cores).

    n_iter > 1 unrolls the whole computation; loop_n > 1 wraps it in a
    hardware For_i loop (both for timing by the slope method); results are
    identical for any value.
    """
    nc = bacc.Bacc("TRN2", target_bir_lowering=False, debug=False,
                   num_devices=N_CORES)

    x_d = nc.dram_tensor("x", [T, C], F32, kind="ExternalInput")
    u_d = nc.dram_tensor("u", [P, P], F32, kind="ExternalInput")
    j_d = nc.dram_tensor("jm", [P, P], F32, kind="ExternalInput")
    r_d = nc.dram_tensor("recip", [P, NB], F32, kind="ExternalInput")
    o_d = nc.dram_tensor("out", [T, C], F32, kind="ExternalOutput")

    with tile.TileContext(nc) as tc:
        with (
            tc.tile_pool(name="consts", bufs=1) as consts,
            tc.tile_pool(name="xin", bufs=3) as xin,
            tc.tile_pool(name="oput", bufs=3) as oput,
            tc.tile_pool(name="zp", bufs=4) as zp,
            tc.tile_pool(name="ps", bufs=8, space="PSUM") as psp,
        ):
            u_t = consts.tile([P, P], F32)
            j_t = consts.tile([P, P], F32)
            r_t = consts.tile([P, NB], F32)
            nc.sync.dma_start(u_t[:], u_d[:])
            nc.sync.dma_start(j_t[:], j_d[:])
            nc.sync.dma_start(r_t[:], r_d[:])

            from contextlib import ExitStack
            loop_ctx = ExitStack()
            if loop_n > 1:
                loop_ctx.enter_context(tc.For_i(0, loop_n, 1))
            for _ in range(n_iter):
                z_prev = None
                for w in range(NW):
                    xw = xin.tile([P, G, C], F32, tag="xw")
                    # rows 1024w..1024(w+1): block j of wave w has rows
                    # (8w+j)*128 + p  ->  AP dims (p, j, c)
                    xv = x_d[w * G * P:(w + 1) * G * P, :].rearrange(
                        "(j p) c -> p j c", p=P)
                    nc.sync.dma_start(xw[:], xv)
                    ow = oput.tile([P, G, C], F32, tag="ow")
                    for j in range(G):
                        k = w * G + j
                        xk = xw[:, j, :]
                        ps = psp.tile([P, C], F32, tag="ps")
                        if k == 0:
                            nc.tensor.matmul(ps[:], u_t[:], xk,
                                             start=True, stop=True)
                        else:
                            nc.tensor.matmul(ps[:], u_t[:], xk,
                                             start=True, stop=False)
                            nc.tensor.matmul(ps[:], j_t[:], z_prev[:],
                                             start=False, stop=True)
                        # running block sum Z_k = Z_{k-1} + X_k
                        if k < NB - 1:
                            z_new = zp.tile([P, C], F32, tag="z")
                            if k == 0:
                                nc.vector.tensor_copy(z_new[:], xk)
                            else:
                                nc.vector.tensor_add(z_new[:], z_prev[:], xk)
                            z_prev = z_new
                        # evacuate PSUM with the 1/(t+1) scale
                        if j % 2 == 0:
                            nc.vector.tensor_scalar_mul(
                                ow[:, j, :], ps[:], r_t[:, k:k + 1])
                        else:
                            nc.scalar.activation(
                                ow[:, j, :], ps[:],
                                mybir.ActivationFunctionType.Copy,
                                scale=r_t[:, k:k + 1])
                    ov = o_d[w * G * P:(w + 1) * G * P, :].rearrange(
                        "(j p) c -> p j c", p=P)
                    nc.scalar.dma_start(ov, ow[:])
            loop_ctx.close()

    nc.compile()
    return nc


def make_consts():
    s = np.arange(P)
    u = (s[:, None] <= s[None, :]).astype(np.float32)          # u[s,t]=1 if s<=t
    jm = np.ones((P, P), dtype=np.float32)
    counts = (np.arange(NB)[None, :] * P + s[:, None] + 1)     # [P, NB]
    recip = (1.0 / counts).astype(np.float32)
    return u, jm, recip


def kernel(x):
    x = np.ascontiguousarray(np.asarray(x), dtype=np.float32)
    assert x.shape == (B, T, C), x.shape
    if "nc" not in _cache:
        _cache["nc"] = build_program()
    nc = _cache["nc"]
    u, jm, recip = make_consts()
    in_maps = [{"x": x[b], "u": u, "jm": jm, "recip": recip}
               for b in range(N_CORES)]
    res = run_bass_kernel_spmd(nc, in_maps, list(range(N_CORES)))
    out = np.stack([res.results[b]["out"] for b in range(N_CORES)], axis=0)
    return out.astype(np.float32, copy=False)


# revision 15
# speedup vs baseline: 1.2060x; 1.2060x over previous
"""Causal bag-of-words kernel for Trainium2 (8 NeuronCores, SPMD).

out[b, t, :] = mean(x[b, :t+1, :], axis=0)  for x of shape (8, 8192, 512) f32.

Sharding: data-parallel over B — core b handles x[b] (8192, 512) independently.

Per-core algorithm (all in natural [t, c] layout, no transposes):
  T = 8192 is split into 64 blocks of 128 rows (partition dim).
  For block k with rows X_k [128, 512]:
    psum_k = U @ X_k + J @ Z_{k-1}      (two accumulating PE matmuls)
  where U is upper-triangular ones (cumsum within the block), J is all-ones
  (broadcasts the column-sum of Z over all 128 rows), and
  Z_{k-1} = sum_{j<k} X_j is a running elementwise block sum maintained with
  one DVE add per block.  The 1/(t+1) scaling is folded into the PSUM->SBUF
  evacuation copy as a per-partition scalar multiply (split DVE/ACT).
  Blocks are streamed in waves of 8 (2 MiB DMAs) and written back the same way.
"""

import sys

sys.path.insert(0, "/opt/trn_rl_repo")

import numpy as np

import concourse.bacc as bacc
import concourse.bass as bass
import concourse.mybir as mybir
import concourse.tile as tile
from concourse.bass_utils import run_bass_kernel_spmd

B, T, C = 8, 8192, 512
P = 128                 # partition dim / block size along T
NB = T // P             # 64 blocks
G = 8                   # blocks per wave (2 MiB per DMA)
NW = NB // G            # 8 waves
N_CORES = 8
F32 = mybir.dt.float32
F32R = mybir.dt.float32r  # full-rate fp32 matmul path (4x faster at N>=256)

_cache: dict = {}


def build_program(n_iter: int = 1, loop_n: int = 1):
    """Build + compile the per-core Bass program (SPMD, identical on all cores).

    n_iter > 1 unrolls the whole computation; loop_n > 1 wraps it in a
    hardware For_i loop (both for timing by the slope method); results are
    identical for any value.
    """
    nc = bacc.Bacc("TRN2", target_bir_lowering=False, debug=False,
                   num_devices=N_CORES)

    x_d = nc.dram_tensor("x", [T, C], F32, kind="ExternalInput")
    u_d = nc.dram_tensor("u", [P, P], F32, kind="ExternalInput")
    j_d = nc.dram_tensor("jm", [P, P], F32, kind="ExternalInput")
    r_d = nc.dram_tensor("recip", [P, NB], F32, kind="ExternalInput")
    o_d = nc.dram_tensor("out", [T, C], F32, kind="ExternalOutput")

    ACT_COPY = mybir.ActivationFunctionType.Copy
    with tile.TileContext(nc) as tc:
        with (
            tc.tile_pool(name="consts", bufs=1) as consts,
            tc.tile_pool(name="xin", bufs=4) as xin,
            tc.tile_pool(name="oput", bufs=3) as oput,
            tc.tile_pool(name="zp", bufs=6) as zp,
            tc.tile_pool(name="zhp", bufs=4) as zhp,
            tc.tile_pool(name="zlp", bufs=4) as zlp,
            tc.tile_pool(name="ps", bufs=8, space="PSUM") as psp,
        ):
            # consts go via SWDGE (gpsimd) so the HWDGE rings start on the
            # first wave load immediately
            u_t = consts.tile([P, P], F32)
            j_t = consts.tile([P, P], F32)
            r_t = consts.tile([P, NB], F32)
            nc.gpsimd.dma_start(u_t[:], u_d[:])
            nc.gpsimd.dma_start(j_t[:], j_d[:])
            nc.gpsimd.dma_start(r_t[:], r_d[:])
            # ones matrix rounded to fp32r (exact) for the full-rate carry MMs
            j_r = consts.tile([P, P], F32R)
            nc.vector.tensor_copy(j_r[:], j_t[:])

            from contextlib import ExitStack
            loop_ctx = ExitStack()
            if loop_n > 1:
                loop_ctx.enter_context(tc.For_i(0, loop_n, 1))
            H = G // 2          # half-wave (1 MiB DMA granularity)
            for _ in range(n_iter):
                z_prev = None
                for w in range(NW):
                    xw = xin.tile([P, G, C], F32, tag="xw")
                    # rows 1024w..1024(w+1): block j of wave w has rows
                    # (8w+j)*128 + p  ->  AP dims (p, j, c)
                    xv = x_d[w * G * P:(w + 1) * G * P, :].rearrange(
                        "(j p) c -> p j c", p=P)
                    if w == 0:
                        # quarter the first load so PE starts sooner
                        for q in range(4):
                            nc.sync.dma_start(xw[:, 2 * q:2 * q + 2, :],
                                              xv[:, 2 * q:2 * q + 2, :])
                    else:
                        nc.sync.dma_start(xw[:], xv)
                    ow = oput.tile([P, G, C], F32, tag="ow")
                    for j in range(G):
                        k = w * G + j
                        xk = xw[:, j, :]
                        ps = psp.tile([P, C], F32, tag="ps")
                        if k == 0:
                            nc.tensor.matmul(ps[:], u_t[:], xk,
                                             start=True, stop=True)
                        else:
                            # carry = J @ Z_{k-1}, exact via fp32r hi+lo:
                            # zh = round_fp32r(z), zl = round_fp32r(z - zh)
                            zh = zhp.tile([P, C], F32R, tag="zh")
                            nc.scalar.activation(zh[:], z_prev[:], ACT_COPY)
                            zl = zlp.tile([P, C], F32R, tag="zl")
                            nc.gpsimd.tensor_sub(zl[:], z_prev[:],
                                                 zh[:].bitcast(F32))
                            nc.tensor.matmul(ps[:], u_t[:], xk,
                                             start=True, stop=False)
                            nc.tensor.matmul(ps[:], j_r[:], zh[:],
                                             start=False, stop=False)
                            nc.tensor.matmul(ps[:], j_r[:], zl[:],
                                             start=False, stop=True)
                        # running block sum Z_k = Z_{k-1} + X_k
                        if k < NB - 1:
                            z_new = zp.tile([P, C], F32, tag="z")
                            if k == 0:
                                nc.vector.tensor_copy(z_new[:], xk)
                            else:
                                nc.vector.tensor_add(z_new[:], z_prev[:], xk)
                            z_prev = z_new
                        # evacuate PSUM with the 1/(t+1) scale
                        nc.vector.tensor_scalar_mul(
                            ow[:, j, :], ps[:], r_t[:, k:k + 1])
                    ov = o_d[w * G * P:(w + 1) * G * P, :].rearrange(
                        "(j p) c -> p j c", p=P)
                    # store per half-wave so the DMA starts 4 blocks earlier;
                    # quarter the final stores to shorten the drain
                    if w == NW - 1:
                        for q in range(4):
                            nc.scalar.dma_start(ov[:, 2 * q:2 * q + 2, :],
                                                ow[:, 2 * q:2 * q + 2, :])
                    else:
                        nc.scalar.dma_start(ov[:, :H, :], ow[:, :H, :])
                        nc.scalar.dma_start(ov[:, H:, :], ow[:, H:, :])
            loop_ctx.close()

    nc.compile()
    return nc


def make_consts():
    s = np.arange(P)
    u = (s[:, None] <= s[None, :]).astype(np.float32)          # u[s,t]=1 if s<=t
    jm = np.ones((P, P), dtype=np.float32)
    counts = (np.arange(NB)[None, :] * P + s[:, None] + 1)     # [P, NB]
    recip = (1.0 / counts).astype(np.float32)
    return u, jm, recip


def kernel(x):
    x = np.ascontiguousarray(np.asarray(x), dtype=np.float32)
    assert x.shape == (B, T, C), x.shape
    if "nc" not in _cache:
        _cache["nc"] = build_program()
    nc = _cache["nc"]
    u, jm, recip = make_consts()
    in_maps = [{"x": x[b], "u": u, "jm": jm, "recip": recip}
               for b in range(N_CORES)]
    res = run_bass_kernel_spmd(nc, in_maps, list(range(N_CORES)))
    out = np.stack([res.results[b]["out"] for b in range(N_CORES)], axis=0)
    return out.astype(np.float32, copy=False)
